# revision 17
# baseline (speedup 1.0000x reference)
"""3-layer GAT + mean-pool + linear head on 8 Trainium2 NeuronCores.

V2 strategy (dst-partition-aligned edge layout, no dst gather):
  - Host: nodes dealt to 8 cores by in-degree round-robin, then a vectorized
    local-search pass swaps equal-degree nodes between cores to even out each
    destination's in-edge spread across the 4 source table quarters (quarter =
    core pair, forced by the AllGather row order and int16 gather indices).
  - Within a core, nodes sort by in-degree into 128-node slots; an edge lives
    at partition p = its dst's slot position, column j = its rank among that
    (dst, quarter)'s edges.  a_dst therefore becomes a per-partition scalar
    (adcol), eliminating the per-edge dst gather entirely.
  - Per layer a table T[node] = [h | 1 | h@a_s | h@a_d | pad] (bf16, 256B
    rows) is the gather source.  Layer 1's table is precomputed on the host
    and shipped as an input (no AllGather).  Layers 2/3 build their shard
    during the previous layer's epilogue and AllGather it.
  - Per slot: ex = exp(leakyrelu(asrc[src] + adcol[p] + c*attr)) (bf16); per
    edge-column the gathered row [h|1] is scaled by ex (DVE, 4x bf16 mode)
    and accumulated into PSUM [128, 65] by a matmul against a constant
    identity (lhsT), yielding [sum ex*h | sum ex] per dst node.
  - Epilogue: x' = x2 + b with x2 = num/(den+eps); next-layer table rows via
    PE transpose + matmul with packed weights; graph mean-pool via PE with a
    host-built (1/cnt)-weighted graph indicator; host sums per-core partials.
"""

import sys

for _p in ("/opt/trn_rl_repo",):
    if _p not in sys.path:
        sys.path.insert(0, _p)

import numpy as np
import ml_dtypes

import concourse.bacc as bacc
import concourse.bass as bass
import concourse.tile as tile
from concourse import bass_utils, mybir

# Problem constants (hardcoded per spec)
N = 100_000
E = 1_600_000
G = 256
HID = 64
NEG_SLOPE = 0.2
EPS = 1e-16

NCORES = 8
NQ = 4             # src table quarters (int16 gather index limit)
P = 128            # partitions / node-block size
RW = 128           # table row width (bf16 cols) -> 256 bytes
C_ONE = 64         # constant-1 column (denominator rides the matmul)
C_AS = 65          # a_src column
C_AD = 66          # a_dst column
TCOLS = 67         # meaningful table columns
PAD_NEG = -1.0e30  # attr_eff value on pad edges -> ex = exp(-inf) = 0

GB_COLS = 96       # target gathered columns per gather batch
GB_SLOTS = 8       # max slots per gather batch
PIECE = 1024       # max indices per dma_gather instruction (HW SWDGE ring limit)

F32 = mybir.dt.float32
BF16 = mybir.dt.bfloat16
I16 = mybir.dt.int16

BF = ml_dtypes.bfloat16

N_SWEEPS = 12      # quarter-balance local search sweeps


class Prep:
    pass


# ----------------------------------------------------------------------------
# Host-side graph preprocessing
# ----------------------------------------------------------------------------

def _wrap16(flat: np.ndarray) -> np.ndarray:
    """int16 stream -> [128, n/16] wrapped layout (k at [k%16, k//16], x8)."""
    n = flat.shape[0]
    assert n % 16 == 0
    w = flat.reshape(n // 16, 16).T          # [16, n/16]
    return np.tile(w, (8, 1))                # [128, n/16]


def _balance_quarters(src, dst, deg_in):
    """Assign nodes to cores (equal counts, aligned degree profiles) while
    evening out each dst's in-edge spread across the 4 quarters.  Batched
    local search with live counts + re-verification.  Returns (core_of, C)
    where C[n, q] = in-edges of n from quarter q."""
    ceil4 = (deg_in + 3) // 4
    nodes = np.arange(N)
    order = np.argsort(-deg_in, kind="stable")
    core_of = np.empty(N, np.int32)
    core_of[order] = np.arange(N) % NCORES

    eorder = np.argsort(src, kind="stable")
    estarts = np.searchsorted(src[eorder], np.arange(N + 1))
    dst_sorted = dst[eorder]

    C = np.zeros((N, NQ), np.int32)
    np.add.at(C, (dst, core_of[src] // 2), 1)
    for _sw in range(N_SWEEPS):
        q_of = (core_of // 2).astype(np.int64)
        addpen = (C >= ceil4[:, None])
        decgain = (C > ceil4[:, None])
        A = np.zeros((N, NQ), np.int64)
        for q in range(NQ):
            np.add.at(A[:, q], src, addpen[dst, q])
        B = np.zeros(N, np.int64)
        np.add.at(B, src, decgain[dst, q_of[src]])
        A2 = A.copy()
        A2[nodes, q_of] = 10**9
        best = np.argmin(A2, axis=1).astype(np.int64)
        gain = B - A2[nodes, best]
        movers = nodes[gain > 0]
        if len(movers) == 0:
            break
        movers = movers[np.argsort(-gain[movers], kind="stable")]
        applied = 0
        for a0 in range(0, len(movers), 2048):
            ch = movers[a0:a0 + 2048]
            eds = [dst_sorted[estarts[s]:estarts[s + 1]] for s in ch]
            lens = np.array([len(e) for e in eds])
            if lens.sum() == 0:
                continue
            flat = np.concatenate(eds)
            seg = np.repeat(np.arange(len(ch)), lens)
            qcur = (core_of[ch] // 2).astype(np.int64)
            Bv = np.zeros(len(ch), np.int64)
            np.add.at(Bv, seg, (C[flat, qcur[seg]] > ceil4[flat]))
            Av = np.zeros((len(ch), NQ), np.int64)
            for q in range(NQ):
                np.add.at(Av[:, q], seg, (C[flat, q] >= ceil4[flat]))
            Av[np.arange(len(ch)), qcur] = 10**9
            bq = np.argmin(Av, axis=1)
            gv = Bv - Av[np.arange(len(ch)), bq]
            ok = gv > 0
            if not ok.any():
                continue
            ids = ch[ok]
            icur = qcur[ok]
            ibest = bq[ok]
            ideg = deg_in[ids]
            lo = np.minimum(icur, ibest)
            hi = np.maximum(icur, ibest)
            fwd = icur == lo
            key = (ideg * 16 + lo * 4 + hi) * 2 + fwd
            srt = np.argsort(key, kind="stable")
            ids, key, fwd = ids[srt], key[srt], fwd[srt]
            uk, st2 = np.unique(key >> 1, return_index=True)
            st2 = list(st2) + [len(key)]
            swap_a, swap_b = [], []
            for gi in range(len(uk)):
                b0, b1 = st2[gi], st2[gi + 1]
                ids_f = ids[b0:b1][fwd[b0:b1]]
                ids_b = ids[b0:b1][~fwd[b0:b1]]
                m = min(len(ids_f), len(ids_b))
                if m:
                    swap_a.append(ids_f[:m])
                    swap_b.append(ids_b[:m])
            if not swap_a:
                continue
            sa = np.concatenate(swap_a)
            sb = np.concatenate(swap_b)
            oq_a = (core_of[sa] // 2).astype(np.int64)
            oq_b = (core_of[sb] // 2).astype(np.int64)
            tmp = core_of[sa].copy()
            core_of[sa] = core_of[sb]
            core_of[sb] = tmp
            moved = np.concatenate([sa, sb])
            oldq = np.concatenate([oq_a, oq_b])
            newq = np.concatenate([oq_b, oq_a])
            me = [dst_sorted[estarts[s]:estarts[s + 1]] for s in moved]
            ml = np.array([len(x) for x in me])
            if ml.sum():
                mf = np.concatenate(me)
                ms = np.repeat(np.arange(len(moved)), ml)
                np.subtract.at(C, (mf, oldq[ms]), 1)
                np.add.at(C, (mf, newq[ms]), 1)
            applied += len(sa)
        if applied == 0:
            break
    return core_of, C


def preprocess(edge_index: np.ndarray, edge_attr: np.ndarray, batch: np.ndarray,
               c_scal) -> Prep:
    pr = Prep()
    src = edge_index[0].astype(np.int64)
    dst = edge_index[1].astype(np.int64)
    attr = edge_attr[:, 0].astype(np.float32)

    deg_in = np.bincount(dst, minlength=N).astype(np.int64)
    core_of, Cq = _balance_quarters(src, dst, deg_in)

    n_per_core = N // NCORES                      # 12500
    n_slots = (n_per_core + P - 1) // P           # 98
    n_loc = n_slots * P                           # 12544
    n_tab = NCORES * n_loc                        # 100352
    qrows = n_tab // NQ                           # 25088
    assert qrows <= 32767

    # within-core order: max per-quarter in-degree desc, then argmax quarter,
    # then profile — groups nodes whose worst quarter matches into the same
    # slot, which is what drives the per-(slot, quarter) column budget.
    mxq = Cq.max(axis=1)
    amq = np.argmax(Cq, axis=1)
    loc_of = np.empty(N, np.int64)
    for c in range(NCORES):
        ids = np.nonzero(core_of == c)[0]
        key = np.lexsort((ids, -Cq[ids, 3], -Cq[ids, 2], -Cq[ids, 1],
                          -Cq[ids, 0], amq[ids], -mxq[ids]))
        srt = ids[key]
        loc_of[srt] = np.arange(len(srt))
    slot_of = loc_of // P
    p_of = loc_of % P
    new_glob = core_of.astype(np.int64) * n_loc + loc_of

    # per-edge placement
    c_e = core_of[dst]
    s_e = slot_of[dst]
    p_e = p_of[dst]
    q_e = (core_of[src] // 2).astype(np.int64)
    # j = rank within (dst, q)
    ordk = np.lexsort((np.arange(E), q_e, dst))
    ds = dst[ordk]
    qs = q_e[ordk]
    grp = np.empty(E, bool)
    grp[0] = True
    grp[1:] = (ds[1:] != ds[:-1]) | (qs[1:] != qs[:-1])
    gid_idx = np.cumsum(grp) - 1
    first_pos = np.full(gid_idx[-1] + 1, E, np.int64)
    np.minimum.at(first_pos, gid_idx, np.arange(E))
    j_sorted = np.arange(E) - first_pos[gid_idx]
    j_e = np.empty(E, np.int64)
    j_e[ordk] = j_sorted

    # tiles per (slot, quarter): global max over cores & partitions
    tq = np.zeros((n_slots, NQ), np.int64)
    np.maximum.at(tq, (s_e, q_e), j_e + 1)
    for s in range(n_slots):
        if tq[s].sum() == 0:
            tq[s, 0] = 1
    cols_slot = tq.sum(axis=1)                     # [n_slots]
    col_off = np.zeros(n_slots + 1, np.int64)
    col_off[1:] = np.cumsum(cols_slot)
    TTC = int(col_off[-1])                         # total edge-columns

    # gather groups
    groups = []
    cur, cur_cols = [], 0
    for s in range(n_slots):
        if cur and (cur_cols + cols_slot[s] > GB_COLS or len(cur) >= GB_SLOTS):
            groups.append(cur)
            cur, cur_cols = [], 0
        cur.append(s)
        cur_cols += int(cols_slot[s])
    if cur:
        groups.append(cur)

    # per-(group, quarter) stream layout
    n_gb = len(groups)
    cols_gq = np.zeros((n_gb, NQ), np.int64)
    for g, sl in enumerate(groups):
        for q in range(NQ):
            cols_gq[g, q] = sum(int(tq[s, q]) for s in sl)
    # chs[(g, q, s)] = column offset of slot s's q-run inside group g's hs
    chs = {}
    base_gq = {}
    for g, sl in enumerate(groups):
        o = 0
        for q in range(NQ):
            base_gq[(g, q)] = o
            for s in sl:
                chs[(g, q, s)] = o
                o += int(tq[s, q])
    # sidx blob: per (g, q) contiguous ranges of idx columns ([128, cols*8])
    sidx_off = {}
    so = 0
    for g in range(n_gb):
        for q in range(NQ):
            sidx_off[(g, q)] = so
            so += int(cols_gq[g, q]) * 8
    SC = so

    # qoff[s][q] = column offset of q-run inside slot s's local cols
    qoff = np.zeros((n_slots, NQ + 1), np.int64)
    for s in range(n_slots):
        qoff[s, 1:] = np.cumsum(tq[s])

    # ---- build per-core streams & meta
    gb_of_slot = np.empty(n_slots, np.int64)
    for g, sl in enumerate(groups):
        for s in sl:
            gb_of_slot[s] = g
    # stream position of each edge inside its core's sidx flat array:
    # pos = sidx_off[(g, q)]*16 ... in IDX units: idxpos = (base of (g,q) in
    # idx units) + (chs[(g,q,s)] - base_gq[(g,q)] + j)*128 + p
    idx_base_gq = {k: v * 16 for k, v in sidx_off.items()}

    e_g = gb_of_slot[s_e]
    e_base = np.empty(E, np.int64)
    e_chs = np.empty(E, np.int64)
    e_bgq = np.empty(E, np.int64)
    # vectorize via lookup tables
    base_tab = np.zeros((n_gb, NQ), np.int64)
    for (g, q), v in idx_base_gq.items():
        base_tab[g, q] = v
    chs_tab = np.zeros((n_slots, NQ), np.int64)
    bgq_tab = np.zeros((n_gb, NQ), np.int64)
    for (g, q), v in base_gq.items():
        bgq_tab[g, q] = v
    for (g, q, s), v in chs.items():
        chs_tab[s, q] = v
    e_base = base_tab[e_g, q_e]
    e_chs = chs_tab[s_e, q_e]
    e_bgq = bgq_tab[e_g, q_e]
    e_idxpos = e_base + (e_chs - e_bgq + j_e) * P + p_e

    sidx = np.zeros((NCORES, P, SC), np.int16)
    for c in range(NCORES):
        m = c_e == c
        flat = np.zeros(SC * 16, np.int16)
        flat[e_idxpos[m]] = (new_glob[src[m]] - q_e[m] * qrows).astype(np.int16)
        sidx[c] = _wrap16(flat)

    # attr_eff meta per layer: [128, TTC] bf16, position (p, col_off[s]+qoff+j)
    e_col = col_off[s_e] + qoff[s_e, q_e] + j_e
    attr_m = np.zeros((3, NCORES, P, TTC), np.float32)
    attr_m[:] = PAD_NEG
    for c in range(NCORES):
        m = c_e == c
        for li in range(3):
            attr_m[li, c, p_e[m], e_col[m]] = attr[m] * np.float32(c_scal[li])
    # pad positions stay PAD_NEG; but positions beyond a slot's real edges in
    # partitions with fewer edges are also PAD_NEG (init).

    # graph pooling tables
    cnt_g = np.bincount(batch.astype(np.int64), minlength=G).astype(np.float32)
    wg = 1.0 / np.maximum(cnt_g, 1.0)
    g_of = batch.astype(np.int64)
    gidm = np.full((NCORES, n_loc), -1.0, np.float32)
    winv = np.zeros((NCORES, n_loc), np.float32)
    ids = np.arange(N)
    gidm[core_of[ids], loc_of[ids]] = g_of.astype(np.float32)
    winv[core_of[ids], loc_of[ids]] = wg[g_of]
    gidm = gidm.reshape(NCORES, n_slots, P).transpose(0, 2, 1)
    winv = winv.reshape(NCORES, n_slots, P).transpose(0, 2, 1)

    pr.n_slots, pr.n_loc, pr.n_tab, pr.qrows, pr.TTC = \
        n_slots, n_loc, n_tab, qrows, TTC
    pr.tq, pr.cols_slot, pr.col_off, pr.qoff = tq, cols_slot, col_off, qoff
    pr.groups, pr.cols_gq, pr.chs, pr.base_gq, pr.sidx_off, pr.SC = \
        groups, cols_gq, chs, base_gq, sidx_off, SC
    pr.sidx, pr.attr_m = sidx, attr_m
    pr.gid, pr.winv = gidm, winv
    pr.core_of, pr.loc_of, pr.new_glob = core_of, loc_of, new_glob
    return pr


def pack_weights(inputs: dict) -> dict:
    w = {}
    for l in (1, 2, 3):
        W = np.asarray(inputs[f"W{l}"], np.float32)
        a_s = np.asarray(inputs[f"as{l}"], np.float32)
        a_d = np.asarray(inputs[f"ad{l}"], np.float32)
        # x @ wext = [h | 0 | h@a_s | h@a_d]; the 0 col is overwritten with 1
        w[f"wext{l}"] = np.concatenate(
            [W, np.zeros((HID, 1), np.float32),
             (W @ a_s)[:, None], (W @ a_d)[:, None]], axis=1)
        w[f"c{l}"] = float(np.asarray(inputs[f"We{l}"], np.float32)[0]
                           @ np.asarray(inputs[f"ae{l}"], np.float32))
        w[f"brep{l}"] = np.tile(np.asarray(inputs[f"b{l}"], np.float32)[None, :],
                                (P, 1))
    w["wlin"] = np.asarray(inputs["Wlin"], np.float32)
    w["blin"] = float(np.asarray(inputs["blin"], np.float32)[0])
    return w


def host_table1(pr: Prep, w: dict, x: np.ndarray):
    """Layer-1 table [n_tab, RW] bf16 (row order new_glob) + adcol1 [c][P,S]."""
    t = x.astype(np.float32) @ w["wext1"]          # [N, 67]
    t[:, C_ONE] = 1.0
    T1 = np.zeros((pr.n_tab, RW), np.float32)
    T1[pr.new_glob[np.arange(N)], :TCOLS] = t
    adcol = np.zeros((NCORES, pr.n_loc), np.float32)
    adcol[pr.core_of, pr.loc_of] = t[:, C_AD]
    adcol = adcol.reshape(NCORES, pr.n_slots, P).transpose(0, 2, 1)
    return T1.astype(BF), adcol


# ----------------------------------------------------------------------------
# Device program
# ----------------------------------------------------------------------------

def build_program(pr: Prep):
    n_slots, n_loc, n_tab, qrows = pr.n_slots, pr.n_loc, pr.n_tab, pr.qrows
    tq, col_off, qoff = pr.tq, pr.col_off, pr.qoff
    groups, cols_gq, chs, base_gq, sidx_off = \
        pr.groups, pr.cols_gq, pr.chs, pr.base_gq, pr.sidx_off

    nc = bacc.Bacc("TRN2", target_bir_lowering=False, debug=False,
                   num_devices=NCORES)
    rg = [list(range(NCORES))]

    T1_d = nc.dram_tensor("T1", [n_tab, RW], BF16, kind="ExternalInput")
    sidx_d = nc.dram_tensor("sidx", [P, pr.SC], I16, kind="ExternalInput")
    attr_d = [nc.dram_tensor(f"attr{l}", [P, pr.TTC], BF16, kind="ExternalInput")
              for l in (1, 2, 3)]
    adcol1_d = nc.dram_tensor("adcol1", [P, n_slots], F32, kind="ExternalInput")
    gid_d = nc.dram_tensor("gid", [P, n_slots], F32, kind="ExternalInput")
    winv_d = nc.dram_tensor("winv", [P, n_slots], F32, kind="ExternalInput")
    wext_d = [nc.dram_tensor(f"wext{l}", [HID, TCOLS], F32, kind="ExternalInput")
              for l in (2, 3)]
    brep_d = [nc.dram_tensor(f"brep{l}", [P, HID], F32, kind="ExternalInput")
              for l in (1, 2, 3)]
    wlin_d = nc.dram_tensor("wlin", [HID, 1], F32, kind="ExternalInput")
    identb_d = nc.dram_tensor("identb", [P, P], BF16, kind="ExternalInput")
    iotg_d = nc.dram_tensor("iotg", [P, G], F32, kind="ExternalInput")
    ident_d = nc.dram_tensor("ident", [P, P], F32, kind="ExternalInput")
    out_d = nc.dram_tensor("out", [P, G // P], F32, kind="ExternalOutput")

    T_full = [None,
              nc.dram_tensor("T2", [n_tab, RW], BF16, kind="Internal",
                             addr_space="Shared"),
              nc.dram_tensor("T3", [n_tab, RW], BF16, kind="Internal",
                             addr_space="Shared")]
    T_sh = [None,
            nc.dram_tensor("Tsh2", [n_loc, RW], BF16, kind="Internal"),
            nc.dram_tensor("Tsh3", [n_loc, RW], BF16, kind="Internal")]

    with tile.TileContext(nc) as tc:
        with (
            tc.tile_pool(name="const", bufs=1) as cpool,
            tc.tile_pool(name="sbuf", bufs=4) as spool,
            tc.tile_pool(name="rs", bufs=8) as rpool,
            tc.tile_pool(name="gath", bufs=3) as gpool,
            tc.tile_pool(name="psum", bufs=2, space="PSUM") as ppool,
            tc.tile_pool(name="psum1", bufs=1, space="PSUM") as ppoolA,
            tc.tile_pool(name="ppool2", bufs=1, space="PSUM") as ppool1,
        ):
            identb_sb = cpool.tile([P, P], BF16, tag="identb")
            nc.sync.dma_start(out=identb_sb[:], in_=identb_d[:, :])
            ident_sb = cpool.tile([P, P], F32, tag="ident")
            nc.sync.dma_start(out=ident_sb[:], in_=ident_d[:, :])
            wext_sb = []
            for i in range(2):
                t1 = cpool.tile([HID, TCOLS], F32, tag=f"wext{i}", name=f"wext{i}")
                nc.sync.dma_start(out=t1[:], in_=wext_d[i][:, :])
                wext_sb.append(t1)
            brep_sb = []
            for i in range(3):
                t2 = cpool.tile([P, HID], F32, tag=f"brep{i}", name=f"brep{i}")
                nc.sync.dma_start(out=t2[:], in_=brep_d[i][:, :])
                brep_sb.append(t2)
            wlin_sb = cpool.tile([HID, 1], F32, tag="wlin")
            nc.sync.dma_start(out=wlin_sb[:], in_=wlin_d[:, :])
            gid_sb = cpool.tile([P, n_slots], F32, tag="gid")
            nc.sync.dma_start(out=gid_sb[:], in_=gid_d[:, :])
            winv_sb = cpool.tile([P, n_slots], F32, tag="winv")
            nc.sync.dma_start(out=winv_sb[:], in_=winv_d[:, :])
            iotg_sb = cpool.tile([P, G], F32, tag="iotg")
            nc.sync.dma_start(out=iotg_sb[:], in_=iotg_d[:, :])
            adcol_sb = [cpool.tile([P, n_slots], F32, tag=f"adcol{l}",
                                   name=f"adcol{l}") for l in range(3)]
            nc.sync.dma_start(out=adcol_sb[0][:], in_=adcol1_d[:, :])

            pool_ps = [ppool1.tile([P, HID], F32, tag=f"pool{h}", name=f"pool{h}")
                       for h in range(G // P)]

            for l in range(3):
                last = l == 2
                tab = T1_d if l == 0 else T_full[l]
                for g, sl in enumerate(groups):
                    gcols = int(cols_gq[g].sum())
                    hs = gpool.tile([P, gcols * RW], BF16, tag="hs",
                                    name=f"hs_{l}_{g}")
                    hs3 = hs[:].rearrange("p (t c) -> p t c", c=RW)
                    for q in range(NQ):
                        ncq = int(cols_gq[g, q])
                        if ncq == 0:
                            continue
                        o = sidx_off[(g, q)]
                        idx_sb = spool.tile([P, ncq * 8], I16, tag="sidx",
                                            name=f"sidx_{l}_{g}_{q}")
                        nc.sync.dma_start(out=idx_sb[:],
                                          in_=sidx_d[:, o:o + ncq * 8])
                        nidx = ncq * P
                        c0 = base_gq[(g, q)]
                        npieces = (nidx + PIECE - 1) // PIECE
                        per = ((nidx // P + npieces - 1) // npieces)  # cols
                        for pi in range(npieces):
                            ca = pi * per
                            cb = min(ncq, (pi + 1) * per)
                            if cb <= ca:
                                continue
                            nc.gpsimd.dma_gather(
                                out_ap=hs3[:, c0 + ca:c0 + cb, :],
                                in_ap=tab[q * qrows:(q + 1) * qrows, :],
                                idxs_ap=idx_sb[:, ca * 8:cb * 8],
                                num_idxs=(cb - ca) * P,
                                num_idxs_reg=(cb - ca) * P, elem_size=RW)

                    for s in sl:
                        t = int(pr.cols_slot[s])
                        o = int(col_off[s])
                        attr_sb = spool.tile([P, t], BF16, tag="attrm",
                                             name=f"attr_{l}_{s}")
                        nc.sync.dma_start(out=attr_sb[:],
                                          in_=attr_d[l][:, o:o + t])
                        # X = attr_eff + adcol[p] (+ asrc per quarter run)
                        X = spool.tile([P, t], F32, tag="xsum",
                                       name=f"X_{l}_{s}")
                        nc.vector.tensor_scalar(
                            out=X[:], in0=attr_sb[:],
                            scalar1=adcol_sb[l][:, s:s + 1],
                            scalar2=None,
                            op0=mybir.AluOpType.add)
                        for q in range(NQ):
                            nt = int(tq[s, q])
                            if nt == 0:
                                continue
                            cj = chs[(g, q, s)]
                            qo = int(qoff[s, q])
                            asrc_v = hs3[:, cj:cj + nt, C_AS:C_AS + 1] \
                                .rearrange("p t c -> p (t c)")
                            nc.vector.tensor_tensor(
                                out=X[:, qo:qo + nt], in0=X[:, qo:qo + nt],
                                in1=asrc_v, op=mybir.AluOpType.add)
                        alf = spool.tile([P, t], F32, tag="alf",
                                         name=f"alf_{l}_{s}")
                        nc.vector.scalar_tensor_tensor(
                            out=alf[:], in0=X[:], scalar=NEG_SLOPE,
                            in1=X[:], op0=mybir.AluOpType.mult,
                            op1=mybir.AluOpType.max)
                        ex = spool.tile([P, t], F32, tag="ex",
                                        name=f"ex_{l}_{s}")
                        nc.scalar.activation(out=ex[:], in_=alf[:],
                                             func=mybir.ActivationFunctionType.Exp)

                        agg = ppool.tile([P, C_ONE + 1], F32, tag="agg",
                                         name=f"agg_{l}_{s}")
                        nm = 0
                        for q in range(NQ):
                            nt = int(tq[s, q])
                            if nt == 0:
                                continue
                            cj = chs[(g, q, s)]
                            qo = int(qoff[s, q])
                            for k in range(nt):
                                rsc = rpool.tile([P, C_ONE + 1], BF16,
                                                 tag="rsc",
                                                 name=f"rsc_{l}_{s}_{qo + k}")
                                nc.vector.tensor_scalar(
                                    out=rsc[:],
                                    in0=hs3[:, cj + k, 0:C_ONE + 1],
                                    scalar1=ex[:, qo + k:qo + k + 1],
                                    scalar2=None,
                                    op0=mybir.AluOpType.mult)
                                nc.tensor.matmul(
                                    out=agg[:], lhsT=identb_sb[:], rhs=rsc[:],
                                    start=(nm == 0), stop=(nm == t - 1),
                                    skip_group_check=True)
                                nm += 1

                        # epilogue
                        dpe = spool.tile([P, 1], F32, tag="dpe",
                                         name=f"dpe_{l}_{s}")
                        nc.vector.tensor_scalar_add(
                            out=dpe[:], in0=agg[:, C_ONE:C_ONE + 1],
                            scalar1=EPS)
                        rcp = spool.tile([P, 1], F32, tag="rcp",
                                         name=f"rcp_{l}_{s}")
                        nc.vector.reciprocal(out=rcp[:], in_=dpe[:])
                        x2 = spool.tile([P, HID], F32, tag="x2",
                                        name=f"x2_{l}_{s}")
                        nc.scalar.activation(
                            out=x2[:], in_=agg[:, 0:C_ONE],
                            func=mybir.ActivationFunctionType.Copy,
                            scale=rcp[:, 0:1])
                        x2b = spool.tile([P, HID], F32, tag="x2b",
                                         name=f"x2b_{l}_{s}")
                        nc.vector.tensor_tensor(out=x2b[:], in0=x2[:],
                                                in1=brep_sb[l][:],
                                                op=mybir.AluOpType.add)
                        if not last:
                            x3 = spool.tile([P, HID], F32, tag="x3",
                                            name=f"x3_{l}_{s}")
                            nc.scalar.activation(
                                out=x3[:], in_=x2b[:],
                                func=mybir.ActivationFunctionType.Relu)
                            xt_ps = ppoolA.tile([HID, P], F32, tag="xtps")
                            nc.tensor.transpose(out=xt_ps[:], in_=x3[:],
                                                identity=ident_sb[:])
                            xt_sb = spool.tile([HID, P], F32, tag="xtsb",
                                               name=f"xt_{l}_{s}")
                            nc.scalar.copy(out=xt_sb[:], in_=xt_ps[:])
                            tn_ps = ppoolA.tile([P, TCOLS], F32, tag="tps")
                            nc.tensor.matmul(out=tn_ps[:], lhsT=xt_sb[:],
                                             rhs=wext_sb[l][:],
                                             start=True, stop=True)
                            nc.vector.tensor_copy(
                                out=adcol_sb[l + 1][:, s:s + 1],
                                in_=tn_ps[:, C_AD:C_AD + 1])
                            trow = spool.tile([P, RW], BF16, tag="trow",
                                              name=f"trow_{l}_{s}")
                            nc.scalar.copy(out=trow[:, 0:TCOLS], in_=tn_ps[:])
                            nc.vector.memset(trow[:, C_ONE:C_ONE + 1], 1.0)
                            nc.sync.dma_start(
                                out=T_sh[l + 1][s * P:(s + 1) * P, :],
                                in_=trow[:])
                        else:
                            for h in range(G // P):
                                gih = spool.tile([P, P], F32, tag="gih",
                                                 name=f"gi_{s}_{h}")
                                nc.vector.tensor_scalar(
                                    out=gih[:],
                                    in0=iotg_sb[:, h * P:(h + 1) * P],
                                    scalar1=gid_sb[:, s:s + 1],
                                    scalar2=winv_sb[:, s:s + 1],
                                    op0=mybir.AluOpType.is_equal,
                                    op1=mybir.AluOpType.mult)
                                nc.tensor.matmul(
                                    out=pool_ps[h][:], lhsT=gih[:], rhs=x2b[:],
                                    start=(s == 0), stop=(s == n_slots - 1),
                                    skip_group_check=True)

                if not last:
                    nc.gpsimd.collective_compute(
                        "AllGather", mybir.AluOpType.bypass, replica_groups=rg,
                        ins=[T_sh[l + 1].ap().opt()],
                        outs=[T_full[l + 1].ap().opt()])

            # ---- head
            out_sb = spool.tile([P, G // P], F32, tag="outsb")
            for h in range(G // P):
                pool_sb = spool.tile([P, HID], F32, tag="poolsb",
                                     name=f"poolsb{h}")
                nc.vector.tensor_copy(out=pool_sb[:], in_=pool_ps[h][:])
                pt_ps = ppoolA.tile([HID, P], F32, tag="xtps")
                nc.tensor.transpose(out=pt_ps[:], in_=pool_sb[:],
                                    identity=ident_sb[:])
                pt_sb = spool.tile([HID, P], F32, tag="xtsb", name=f"ptsb{h}")
                nc.scalar.copy(out=pt_sb[:], in_=pt_ps[:])
                o_ps = ppoolA.tile([P, 1], F32, tag="tps", name=f"o_ps{h}")
                nc.tensor.matmul(out=o_ps[:], lhsT=pt_sb[:], rhs=wlin_sb[:],
                                 start=True, stop=True)
                nc.vector.tensor_copy(out=out_sb[:, h:h + 1], in_=o_ps[:])
            nc.sync.dma_start(out=out_d[:, :], in_=out_sb[:])

    nc.compile()
    return nc


# ----------------------------------------------------------------------------
# Entry point
# ----------------------------------------------------------------------------

def make_inmaps(pr: Prep, w: dict, T1, adcol1):
    identb = np.eye(P, dtype=np.float32).astype(BF)
    ident = np.eye(P, dtype=np.float32)
    iotg = np.tile(np.arange(G, dtype=np.float32)[None, :], (P, 1))
    in_maps = []
    for c in range(NCORES):
        m = {
            "T1": T1,
            "sidx": pr.sidx[c],
            "adcol1": adcol1[c],
            "gid": pr.gid[c],
            "winv": pr.winv[c],
            "wlin": w["wlin"],
            "identb": identb,
            "ident": ident,
            "iotg": iotg,
        }
        for li, l in enumerate((1, 2, 3)):
            m[f"attr{l}"] = pr.attr_m[li, c].astype(BF)
            m[f"brep{l}"] = w[f"brep{l}"]
        for l in (2, 3):
            m[f"wext{l}"] = w[f"wext{l}"]
        in_maps.append(m)
    return in_maps


def kernel(**inputs) -> np.ndarray:
    inputs = {k: np.asarray(v) for k, v in inputs.items()}
    w = pack_weights(inputs)
    pr = preprocess(inputs["edge_index"], inputs["edge_attr"], inputs["batch"],
                    [w["c1"], w["c2"], w["c3"]])
    T1, adcol1 = host_table1(pr, w, np.asarray(inputs["x"], np.float32))
    nc = build_program(pr)
    in_maps = make_inmaps(pr, w, T1, adcol1)
    res = bass_utils.run_bass_kernel_spmd(nc, in_maps,
                                          core_ids=list(range(NCORES)))
    out = np.zeros(G, np.float64)
    for c in range(NCORES):
        oc = res.results[c]["out"]
        out += oc.T.reshape(-1).astype(np.float64)
    return (out + w["blin"]).astype(np.float32)


# revision 18
# speedup vs baseline: 1.0018x; 1.0018x over previous
"""3-layer GAT + mean-pool + linear head on 8 Trainium2 NeuronCores.

V2 strategy (dst-partition-aligned edge layout, no dst gather):
  - Host: nodes dealt to 8 cores by in-degree round-robin, then a vectorized
    local-search pass swaps equal-degree nodes between cores to even out each
    destination's in-edge spread across the 4 source table quarters (quarter =
    core pair, forced by the AllGather row order and int16 gather indices).
  - Within a core, nodes sort by in-degree into 128-node slots; an edge lives
    at partition p = its dst's slot position, column j = its rank among that
    (dst, quarter)'s edges.  a_dst therefore becomes a per-partition scalar
    (adcol), eliminating the per-edge dst gather entirely.
  - Per layer a table T[node] = [h | 1 | h@a_s | h@a_d | pad] (bf16, 256B
    rows) is the gather source.  Layer 1's table is precomputed on the host
    and shipped as an input (no AllGather).  Layers 2/3 build their shard
    during the previous layer's epilogue and AllGather it.
  - Per slot: ex = exp(leakyrelu(asrc[src] + adcol[p] + c*attr)) (bf16); per
    edge-column the gathered row [h|1] is scaled by ex (DVE, 4x bf16 mode)
    and accumulated into PSUM [128, 65] by a matmul against a constant
    identity (lhsT), yielding [sum ex*h | sum ex] per dst node.
  - Epilogue: x' = x2 + b with x2 = num/(den+eps); next-layer table rows via
    PE transpose + matmul with packed weights; graph mean-pool via PE with a
    host-built (1/cnt)-weighted graph indicator; host sums per-core partials.
"""

import sys

for _p in ("/opt/trn_rl_repo",):
    if _p not in sys.path:
        sys.path.insert(0, _p)

import numpy as np
import ml_dtypes

import concourse.bacc as bacc
import concourse.bass as bass
import concourse.tile as tile
from concourse import bass_utils, mybir

# Problem constants (hardcoded per spec)
N = 100_000
E = 1_600_000
G = 256
HID = 64
NEG_SLOPE = 0.2
EPS = 1e-16

NCORES = 8
NQ = 4             # src table quarters (int16 gather index limit)
P = 128            # partitions / node-block size
RW = 128           # table row width (bf16 cols) -> 256 bytes
C_ONE = 64         # constant-1 column (denominator rides the matmul)
C_AS = 65          # a_src column
C_AD = 66          # a_dst column
TCOLS = 67         # meaningful table columns
PAD_NEG = -1.0e30  # attr_eff value on pad edges -> ex = exp(-inf) = 0

GB_COLS = 96       # target gathered columns per gather batch
GB_SLOTS = 8       # max slots per gather batch
PIECE = 1024       # max indices per dma_gather instruction (HW SWDGE ring limit)

F32 = mybir.dt.float32
BF16 = mybir.dt.bfloat16
I16 = mybir.dt.int16

BF = ml_dtypes.bfloat16

N_SWEEPS = 25      # quarter-balance local search sweeps


class Prep:
    pass


# ----------------------------------------------------------------------------
# Host-side graph preprocessing
# ----------------------------------------------------------------------------

def _wrap16(flat: np.ndarray) -> np.ndarray:
    """int16 stream -> [128, n/16] wrapped layout (k at [k%16, k//16], x8)."""
    n = flat.shape[0]
    assert n % 16 == 0
    w = flat.reshape(n // 16, 16).T          # [16, n/16]
    return np.tile(w, (8, 1))                # [128, n/16]


def _balance_quarters(src, dst, deg_in):
    """Assign nodes to cores (equal counts, aligned degree profiles) while
    evening out each dst's in-edge spread across the 4 quarters.  Batched
    local search with live counts + re-verification.  Returns (core_of, C)
    where C[n, q] = in-edges of n from quarter q."""
    ceil4 = (deg_in + 3) // 4
    nodes = np.arange(N)
    order = np.argsort(-deg_in, kind="stable")
    core_of = np.empty(N, np.int32)
    core_of[order] = np.arange(N) % NCORES

    eorder = np.argsort(src, kind="stable")
    estarts = np.searchsorted(src[eorder], np.arange(N + 1))
    dst_sorted = dst[eorder]

    C = np.zeros((N, NQ), np.int32)
    np.add.at(C, (dst, core_of[src] // 2), 1)
    for _sw in range(N_SWEEPS):
        q_of = (core_of // 2).astype(np.int64)
        addpen = (C >= ceil4[:, None])
        decgain = (C > ceil4[:, None])
        A = np.zeros((N, NQ), np.int64)
        for q in range(NQ):
            np.add.at(A[:, q], src, addpen[dst, q])
        B = np.zeros(N, np.int64)
        np.add.at(B, src, decgain[dst, q_of[src]])
        A2 = A.copy()
        A2[nodes, q_of] = 10**9
        best = np.argmin(A2, axis=1).astype(np.int64)
        gain = B - A2[nodes, best]
        movers = nodes[gain > 0]
        if len(movers) == 0:
            break
        movers = movers[np.argsort(-gain[movers], kind="stable")]
        applied = 0
        for a0 in range(0, len(movers), 2048):
            ch = movers[a0:a0 + 2048]
            eds = [dst_sorted[estarts[s]:estarts[s + 1]] for s in ch]
            lens = np.array([len(e) for e in eds])
            if lens.sum() == 0:
                continue
            flat = np.concatenate(eds)
            seg = np.repeat(np.arange(len(ch)), lens)
            qcur = (core_of[ch] // 2).astype(np.int64)
            Bv = np.zeros(len(ch), np.int64)
            np.add.at(Bv, seg, (C[flat, qcur[seg]] > ceil4[flat]))
            Av = np.zeros((len(ch), NQ), np.int64)
            for q in range(NQ):
                np.add.at(Av[:, q], seg, (C[flat, q] >= ceil4[flat]))
            Av[np.arange(len(ch)), qcur] = 10**9
            bq = np.argmin(Av, axis=1)
            gv = Bv - Av[np.arange(len(ch)), bq]
            ok = gv > 0
            if not ok.any():
                continue
            ids = ch[ok]
            icur = qcur[ok]
            ibest = bq[ok]
            ideg = deg_in[ids]
            lo = np.minimum(icur, ibest)
            hi = np.maximum(icur, ibest)
            fwd = icur == lo
            key = (ideg * 16 + lo * 4 + hi) * 2 + fwd
            srt = np.argsort(key, kind="stable")
            ids, key, fwd = ids[srt], key[srt], fwd[srt]
            uk, st2 = np.unique(key >> 1, return_index=True)
            st2 = list(st2) + [len(key)]
            swap_a, swap_b = [], []
            for gi in range(len(uk)):
                b0, b1 = st2[gi], st2[gi + 1]
                ids_f = ids[b0:b1][fwd[b0:b1]]
                ids_b = ids[b0:b1][~fwd[b0:b1]]
                m = min(len(ids_f), len(ids_b))
                if m:
                    swap_a.append(ids_f[:m])
                    swap_b.append(ids_b[:m])
            if not swap_a:
                continue
            sa = np.concatenate(swap_a)
            sb = np.concatenate(swap_b)
            oq_a = (core_of[sa] // 2).astype(np.int64)
            oq_b = (core_of[sb] // 2).astype(np.int64)
            tmp = core_of[sa].copy()
            core_of[sa] = core_of[sb]
            core_of[sb] = tmp
            moved = np.concatenate([sa, sb])
            oldq = np.concatenate([oq_a, oq_b])
            newq = np.concatenate([oq_b, oq_a])
            me = [dst_sorted[estarts[s]:estarts[s + 1]] for s in moved]
            ml = np.array([len(x) for x in me])
            if ml.sum():
                mf = np.concatenate(me)
                ms = np.repeat(np.arange(len(moved)), ml)
                np.subtract.at(C, (mf, oldq[ms]), 1)
                np.add.at(C, (mf, newq[ms]), 1)
            applied += len(sa)
        if applied == 0:
            break
    return core_of, C


def preprocess(edge_index: np.ndarray, edge_attr: np.ndarray, batch: np.ndarray,
               c_scal) -> Prep:
    pr = Prep()
    src = edge_index[0].astype(np.int64)
    dst = edge_index[1].astype(np.int64)
    attr = edge_attr[:, 0].astype(np.float32)

    deg_in = np.bincount(dst, minlength=N).astype(np.int64)
    core_of, Cq = _balance_quarters(src, dst, deg_in)

    n_per_core = N // NCORES                      # 12500
    n_slots = (n_per_core + P - 1) // P           # 98
    n_loc = n_slots * P                           # 12544
    n_tab = NCORES * n_loc                        # 100352
    qrows = n_tab // NQ                           # 25088
    assert qrows <= 32767

    # within-core order: max per-quarter in-degree desc, then argmax quarter,
    # then profile — groups nodes whose worst quarter matches into the same
    # slot, which is what drives the per-(slot, quarter) column budget.
    mxq = Cq.max(axis=1)
    amq = np.argmax(Cq, axis=1)
    loc_of = np.empty(N, np.int64)
    for c in range(NCORES):
        ids = np.nonzero(core_of == c)[0]
        key = np.lexsort((ids, -Cq[ids, 3], -Cq[ids, 2], -Cq[ids, 1],
                          -Cq[ids, 0], amq[ids], -mxq[ids]))
        srt = ids[key]
        loc_of[srt] = np.arange(len(srt))
    slot_of = loc_of // P
    p_of = loc_of % P
    new_glob = core_of.astype(np.int64) * n_loc + loc_of

    # per-edge placement
    c_e = core_of[dst]
    s_e = slot_of[dst]
    p_e = p_of[dst]
    q_e = (core_of[src] // 2).astype(np.int64)
    # j = rank within (dst, q)
    ordk = np.lexsort((np.arange(E), q_e, dst))
    ds = dst[ordk]
    qs = q_e[ordk]
    grp = np.empty(E, bool)
    grp[0] = True
    grp[1:] = (ds[1:] != ds[:-1]) | (qs[1:] != qs[:-1])
    gid_idx = np.cumsum(grp) - 1
    first_pos = np.full(gid_idx[-1] + 1, E, np.int64)
    np.minimum.at(first_pos, gid_idx, np.arange(E))
    j_sorted = np.arange(E) - first_pos[gid_idx]
    j_e = np.empty(E, np.int64)
    j_e[ordk] = j_sorted

    # tiles per (slot, quarter): global max over cores & partitions
    tq = np.zeros((n_slots, NQ), np.int64)
    np.maximum.at(tq, (s_e, q_e), j_e + 1)
    for s in range(n_slots):
        if tq[s].sum() == 0:
            tq[s, 0] = 1
    cols_slot = tq.sum(axis=1)                     # [n_slots]
    col_off = np.zeros(n_slots + 1, np.int64)
    col_off[1:] = np.cumsum(cols_slot)
    TTC = int(col_off[-1])                         # total edge-columns

    # gather groups
    groups = []
    cur, cur_cols = [], 0
    for s in range(n_slots):
        if cur and (cur_cols + cols_slot[s] > GB_COLS or len(cur) >= GB_SLOTS):
            groups.append(cur)
            cur, cur_cols = [], 0
        cur.append(s)
        cur_cols += int(cols_slot[s])
    if cur:
        groups.append(cur)

    # per-(group, quarter) stream layout
    n_gb = len(groups)
    cols_gq = np.zeros((n_gb, NQ), np.int64)
    for g, sl in enumerate(groups):
        for q in range(NQ):
            cols_gq[g, q] = sum(int(tq[s, q]) for s in sl)
    # chs[(g, q, s)] = column offset of slot s's q-run inside group g's hs
    chs = {}
    base_gq = {}
    for g, sl in enumerate(groups):
        o = 0
        for q in range(NQ):
            base_gq[(g, q)] = o
            for s in sl:
                chs[(g, q, s)] = o
                o += int(tq[s, q])
    # sidx blob: per (g, q) contiguous ranges of idx columns ([128, cols*8])
    sidx_off = {}
    so = 0
    for g in range(n_gb):
        for q in range(NQ):
            sidx_off[(g, q)] = so
            so += int(cols_gq[g, q]) * 8
    SC = so

    # qoff[s][q] = column offset of q-run inside slot s's local cols
    qoff = np.zeros((n_slots, NQ + 1), np.int64)
    for s in range(n_slots):
        qoff[s, 1:] = np.cumsum(tq[s])

    # ---- build per-core streams & meta
    gb_of_slot = np.empty(n_slots, np.int64)
    for g, sl in enumerate(groups):
        for s in sl:
            gb_of_slot[s] = g
    # stream position of each edge inside its core's sidx flat array:
    # pos = sidx_off[(g, q)]*16 ... in IDX units: idxpos = (base of (g,q) in
    # idx units) + (chs[(g,q,s)] - base_gq[(g,q)] + j)*128 + p
    idx_base_gq = {k: v * 16 for k, v in sidx_off.items()}

    e_g = gb_of_slot[s_e]
    e_base = np.empty(E, np.int64)
    e_chs = np.empty(E, np.int64)
    e_bgq = np.empty(E, np.int64)
    # vectorize via lookup tables
    base_tab = np.zeros((n_gb, NQ), np.int64)
    for (g, q), v in idx_base_gq.items():
        base_tab[g, q] = v
    chs_tab = np.zeros((n_slots, NQ), np.int64)
    bgq_tab = np.zeros((n_gb, NQ), np.int64)
    for (g, q), v in base_gq.items():
        bgq_tab[g, q] = v
    for (g, q, s), v in chs.items():
        chs_tab[s, q] = v
    e_base = base_tab[e_g, q_e]
    e_chs = chs_tab[s_e, q_e]
    e_bgq = bgq_tab[e_g, q_e]
    e_idxpos = e_base + (e_chs - e_bgq + j_e) * P + p_e

    sidx = np.zeros((NCORES, P, SC), np.int16)
    for c in range(NCORES):
        m = c_e == c
        flat = np.zeros(SC * 16, np.int16)
        flat[e_idxpos[m]] = (new_glob[src[m]] - q_e[m] * qrows).astype(np.int16)
        sidx[c] = _wrap16(flat)

    # attr_eff meta per layer: [128, TTC] bf16, position (p, col_off[s]+qoff+j)
    e_col = col_off[s_e] + qoff[s_e, q_e] + j_e
    attr_m = np.zeros((3, NCORES, P, TTC), np.float32)
    attr_m[:] = PAD_NEG
    for c in range(NCORES):
        m = c_e == c
        for li in range(3):
            attr_m[li, c, p_e[m], e_col[m]] = attr[m] * np.float32(c_scal[li])
    # pad positions stay PAD_NEG; but positions beyond a slot's real edges in
    # partitions with fewer edges are also PAD_NEG (init).

    # graph pooling tables
    cnt_g = np.bincount(batch.astype(np.int64), minlength=G).astype(np.float32)
    wg = 1.0 / np.maximum(cnt_g, 1.0)
    g_of = batch.astype(np.int64)
    gidm = np.full((NCORES, n_loc), -1.0, np.float32)
    winv = np.zeros((NCORES, n_loc), np.float32)
    ids = np.arange(N)
    gidm[core_of[ids], loc_of[ids]] = g_of.astype(np.float32)
    winv[core_of[ids], loc_of[ids]] = wg[g_of]
    gidm = gidm.reshape(NCORES, n_slots, P).transpose(0, 2, 1)
    winv = winv.reshape(NCORES, n_slots, P).transpose(0, 2, 1)

    pr.n_slots, pr.n_loc, pr.n_tab, pr.qrows, pr.TTC = \
        n_slots, n_loc, n_tab, qrows, TTC
    pr.tq, pr.cols_slot, pr.col_off, pr.qoff = tq, cols_slot, col_off, qoff
    pr.groups, pr.cols_gq, pr.chs, pr.base_gq, pr.sidx_off, pr.SC = \
        groups, cols_gq, chs, base_gq, sidx_off, SC
    pr.sidx, pr.attr_m = sidx, attr_m
    pr.gid, pr.winv = gidm, winv
    pr.core_of, pr.loc_of, pr.new_glob = core_of, loc_of, new_glob
    return pr


def pack_weights(inputs: dict) -> dict:
    w = {}
    for l in (1, 2, 3):
        W = np.asarray(inputs[f"W{l}"], np.float32)
        a_s = np.asarray(inputs[f"as{l}"], np.float32)
        a_d = np.asarray(inputs[f"ad{l}"], np.float32)
        # x @ wext = [h | 0 | h@a_s | h@a_d]; the 0 col is overwritten with 1
        w[f"wext{l}"] = np.concatenate(
            [W, np.zeros((HID, 1), np.float32),
             (W @ a_s)[:, None], (W @ a_d)[:, None]], axis=1)
        w[f"c{l}"] = float(np.asarray(inputs[f"We{l}"], np.float32)[0]
                           @ np.asarray(inputs[f"ae{l}"], np.float32))
        w[f"brep{l}"] = np.tile(np.asarray(inputs[f"b{l}"], np.float32)[None, :],
                                (P, 1))
    w["wlin"] = np.asarray(inputs["Wlin"], np.float32)
    w["blin"] = float(np.asarray(inputs["blin"], np.float32)[0])
    return w


def host_table1(pr: Prep, w: dict, x: np.ndarray):
    """Layer-1 table [n_tab, RW] bf16 (row order new_glob) + adcol1 [c][P,S]."""
    t = x.astype(np.float32) @ w["wext1"]          # [N, 67]
    t[:, C_ONE] = 1.0
    T1 = np.zeros((pr.n_tab, RW), np.float32)
    T1[pr.new_glob[np.arange(N)], :TCOLS] = t
    adcol = np.zeros((NCORES, pr.n_loc), np.float32)
    adcol[pr.core_of, pr.loc_of] = t[:, C_AD]
    adcol = adcol.reshape(NCORES, pr.n_slots, P).transpose(0, 2, 1)
    return T1.astype(BF), adcol


# ----------------------------------------------------------------------------
# Device program
# ----------------------------------------------------------------------------

def build_program(pr: Prep):
    n_slots, n_loc, n_tab, qrows = pr.n_slots, pr.n_loc, pr.n_tab, pr.qrows
    tq, col_off, qoff = pr.tq, pr.col_off, pr.qoff
    groups, cols_gq, chs, base_gq, sidx_off = \
        pr.groups, pr.cols_gq, pr.chs, pr.base_gq, pr.sidx_off

    nc = bacc.Bacc("TRN2", target_bir_lowering=False, debug=False,
                   num_devices=NCORES)
    rg = [list(range(NCORES))]

    T1_d = nc.dram_tensor("T1", [n_tab, RW], BF16, kind="ExternalInput")
    sidx_d = nc.dram_tensor("sidx", [P, pr.SC], I16, kind="ExternalInput")
    attr_d = [nc.dram_tensor(f"attr{l}", [P, pr.TTC], BF16, kind="ExternalInput")
              for l in (1, 2, 3)]
    adcol1_d = nc.dram_tensor("adcol1", [P, n_slots], F32, kind="ExternalInput")
    gid_d = nc.dram_tensor("gid", [P, n_slots], F32, kind="ExternalInput")
    winv_d = nc.dram_tensor("winv", [P, n_slots], F32, kind="ExternalInput")
    wext_d = [nc.dram_tensor(f"wext{l}", [HID, TCOLS], F32, kind="ExternalInput")
              for l in (2, 3)]
    brep_d = [nc.dram_tensor(f"brep{l}", [P, HID], F32, kind="ExternalInput")
              for l in (1, 2, 3)]
    wlin_d = nc.dram_tensor("wlin", [HID, 1], F32, kind="ExternalInput")
    identb_d = nc.dram_tensor("identb", [P, P], BF16, kind="ExternalInput")
    iotg_d = nc.dram_tensor("iotg", [P, G], F32, kind="ExternalInput")
    ident_d = nc.dram_tensor("ident", [P, P], F32, kind="ExternalInput")
    out_d = nc.dram_tensor("out", [P, G // P], F32, kind="ExternalOutput")

    T_full = [None,
              nc.dram_tensor("T2", [n_tab, RW], BF16, kind="Internal",
                             addr_space="Shared"),
              nc.dram_tensor("T3", [n_tab, RW], BF16, kind="Internal",
                             addr_space="Shared")]
    T_sh = [None,
            nc.dram_tensor("Tsh2", [n_loc, RW], BF16, kind="Internal"),
            nc.dram_tensor("Tsh3", [n_loc, RW], BF16, kind="Internal")]

    with tile.TileContext(nc) as tc:
        with (
            tc.tile_pool(name="const", bufs=1) as cpool,
            tc.tile_pool(name="sbuf", bufs=4) as spool,
            tc.tile_pool(name="rs", bufs=8) as rpool,
            tc.tile_pool(name="gath", bufs=3) as gpool,
            tc.tile_pool(name="psum", bufs=2, space="PSUM") as ppool,
            tc.tile_pool(name="psum1", bufs=1, space="PSUM") as ppoolA,
            tc.tile_pool(name="ppool2", bufs=1, space="PSUM") as ppool1,
        ):
            identb_sb = cpool.tile([P, P], BF16, tag="identb")
            nc.sync.dma_start(out=identb_sb[:], in_=identb_d[:, :])
            ident_sb = cpool.tile([P, P], F32, tag="ident")
            nc.sync.dma_start(out=ident_sb[:], in_=ident_d[:, :])
            wext_sb = []
            for i in range(2):
                t1 = cpool.tile([HID, TCOLS], F32, tag=f"wext{i}", name=f"wext{i}")
                nc.sync.dma_start(out=t1[:], in_=wext_d[i][:, :])
                wext_sb.append(t1)
            brep_sb = []
            for i in range(3):
                t2 = cpool.tile([P, HID], F32, tag=f"brep{i}", name=f"brep{i}")
                nc.sync.dma_start(out=t2[:], in_=brep_d[i][:, :])
                brep_sb.append(t2)
            wlin_sb = cpool.tile([HID, 1], F32, tag="wlin")
            nc.sync.dma_start(out=wlin_sb[:], in_=wlin_d[:, :])
            gid_sb = cpool.tile([P, n_slots], F32, tag="gid")
            nc.sync.dma_start(out=gid_sb[:], in_=gid_d[:, :])
            winv_sb = cpool.tile([P, n_slots], F32, tag="winv")
            nc.sync.dma_start(out=winv_sb[:], in_=winv_d[:, :])
            iotg_sb = cpool.tile([P, G], F32, tag="iotg")
            nc.sync.dma_start(out=iotg_sb[:], in_=iotg_d[:, :])
            adcol_sb = [cpool.tile([P, n_slots], F32, tag=f"adcol{l}",
                                   name=f"adcol{l}") for l in range(3)]
            nc.sync.dma_start(out=adcol_sb[0][:], in_=adcol1_d[:, :])

            pool_ps = [ppool1.tile([P, HID], F32, tag=f"pool{h}", name=f"pool{h}")
                       for h in range(G // P)]

            for l in range(3):
                last = l == 2
                tab = T1_d if l == 0 else T_full[l]
                for g, sl in enumerate(groups):
                    gcols = int(cols_gq[g].sum())
                    hs = gpool.tile([P, gcols * RW], BF16, tag="hs",
                                    name=f"hs_{l}_{g}")
                    hs3 = hs[:].rearrange("p (t c) -> p t c", c=RW)
                    for q in range(NQ):
                        ncq = int(cols_gq[g, q])
                        if ncq == 0:
                            continue
                        o = sidx_off[(g, q)]
                        idx_sb = spool.tile([P, ncq * 8], I16, tag="sidx",
                                            name=f"sidx_{l}_{g}_{q}")
                        nc.sync.dma_start(out=idx_sb[:],
                                          in_=sidx_d[:, o:o + ncq * 8])
                        nidx = ncq * P
                        c0 = base_gq[(g, q)]
                        npieces = (nidx + PIECE - 1) // PIECE
                        per = ((nidx // P + npieces - 1) // npieces)  # cols
                        for pi in range(npieces):
                            ca = pi * per
                            cb = min(ncq, (pi + 1) * per)
                            if cb <= ca:
                                continue
                            nc.gpsimd.dma_gather(
                                out_ap=hs3[:, c0 + ca:c0 + cb, :],
                                in_ap=tab[q * qrows:(q + 1) * qrows, :],
                                idxs_ap=idx_sb[:, ca * 8:cb * 8],
                                num_idxs=(cb - ca) * P,
                                num_idxs_reg=(cb - ca) * P, elem_size=RW)

                    for s in sl:
                        t = int(pr.cols_slot[s])
                        o = int(col_off[s])
                        attr_sb = spool.tile([P, t], BF16, tag="attrm",
                                             name=f"attr_{l}_{s}")
                        nc.sync.dma_start(out=attr_sb[:],
                                          in_=attr_d[l][:, o:o + t])
                        # X = attr_eff + adcol[p] (+ asrc per quarter run)
                        X = spool.tile([P, t], F32, tag="xsum",
                                       name=f"X_{l}_{s}")
                        nc.vector.tensor_scalar(
                            out=X[:], in0=attr_sb[:],
                            scalar1=adcol_sb[l][:, s:s + 1],
                            scalar2=None,
                            op0=mybir.AluOpType.add)
                        for q in range(NQ):
                            nt = int(tq[s, q])
                            if nt == 0:
                                continue
                            cj = chs[(g, q, s)]
                            qo = int(qoff[s, q])
                            asrc_v = hs3[:, cj:cj + nt, C_AS:C_AS + 1] \
                                .rearrange("p t c -> p (t c)")
                            nc.vector.tensor_tensor(
                                out=X[:, qo:qo + nt], in0=X[:, qo:qo + nt],
                                in1=asrc_v, op=mybir.AluOpType.add)
                        alf = spool.tile([P, t], F32, tag="alf",
                                         name=f"alf_{l}_{s}")
                        nc.vector.scalar_tensor_tensor(
                            out=alf[:], in0=X[:], scalar=NEG_SLOPE,
                            in1=X[:], op0=mybir.AluOpType.mult,
                            op1=mybir.AluOpType.max)
                        ex = spool.tile([P, t], F32, tag="ex",
                                        name=f"ex_{l}_{s}")
                        nc.scalar.activation(out=ex[:], in_=alf[:],
                                             func=mybir.ActivationFunctionType.Exp)

                        agg = ppool.tile([P, C_ONE + 1], F32, tag="agg",
                                         name=f"agg_{l}_{s}")
                        nm = 0
                        for q in range(NQ):
                            nt = int(tq[s, q])
                            if nt == 0:
                                continue
                            cj = chs[(g, q, s)]
                            qo = int(qoff[s, q])
                            for k in range(nt):
                                rsc = rpool.tile([P, C_ONE + 1], BF16,
                                                 tag="rsc",
                                                 name=f"rsc_{l}_{s}_{qo + k}")
                                nc.vector.tensor_scalar(
                                    out=rsc[:],
                                    in0=hs3[:, cj + k, 0:C_ONE + 1],
                                    scalar1=ex[:, qo + k:qo + k + 1],
                                    scalar2=None,
                                    op0=mybir.AluOpType.mult)
                                nc.tensor.matmul(
                                    out=agg[:], lhsT=identb_sb[:], rhs=rsc[:],
                                    start=(nm == 0), stop=(nm == t - 1),
                                    skip_group_check=True)
                                nm += 1

                        # epilogue
                        dpe = spool.tile([P, 1], F32, tag="dpe",
                                         name=f"dpe_{l}_{s}")
                        nc.vector.tensor_scalar_add(
                            out=dpe[:], in0=agg[:, C_ONE:C_ONE + 1],
                            scalar1=EPS)
                        rcp = spool.tile([P, 1], F32, tag="rcp",
                                         name=f"rcp_{l}_{s}")
                        nc.vector.reciprocal(out=rcp[:], in_=dpe[:])
                        x2 = spool.tile([P, HID], F32, tag="x2",
                                        name=f"x2_{l}_{s}")
                        nc.scalar.activation(
                            out=x2[:], in_=agg[:, 0:C_ONE],
                            func=mybir.ActivationFunctionType.Copy,
                            scale=rcp[:, 0:1])
                        x2b = spool.tile([P, HID], F32, tag="x2b",
                                         name=f"x2b_{l}_{s}")
                        nc.vector.tensor_tensor(out=x2b[:], in0=x2[:],
                                                in1=brep_sb[l][:],
                                                op=mybir.AluOpType.add)
                        if not last:
                            x3 = spool.tile([P, HID], F32, tag="x3",
                                            name=f"x3_{l}_{s}")
                            nc.scalar.activation(
                                out=x3[:], in_=x2b[:],
                                func=mybir.ActivationFunctionType.Relu)
                            xt_ps = ppoolA.tile([HID, P], F32, tag="xtps")
                            nc.tensor.transpose(out=xt_ps[:], in_=x3[:],
                                                identity=ident_sb[:])
                            xt_sb = spool.tile([HID, P], F32, tag="xtsb",
                                               name=f"xt_{l}_{s}")
                            nc.scalar.copy(out=xt_sb[:], in_=xt_ps[:])
                            tn_ps = ppoolA.tile([P, TCOLS], F32, tag="tps")
                            nc.tensor.matmul(out=tn_ps[:], lhsT=xt_sb[:],
                                             rhs=wext_sb[l][:],
                                             start=True, stop=True)
                            nc.vector.tensor_copy(
                                out=adcol_sb[l + 1][:, s:s + 1],
                                in_=tn_ps[:, C_AD:C_AD + 1])
                            trow = spool.tile([P, RW], BF16, tag="trow",
                                              name=f"trow_{l}_{s}")
                            nc.scalar.copy(out=trow[:, 0:TCOLS], in_=tn_ps[:])
                            nc.vector.memset(trow[:, C_ONE:C_ONE + 1], 1.0)
                            nc.sync.dma_start(
                                out=T_sh[l + 1][s * P:(s + 1) * P, :],
                                in_=trow[:])
                        else:
                            for h in range(G // P):
                                gih = spool.tile([P, P], F32, tag="gih",
                                                 name=f"gi_{s}_{h}")
                                nc.vector.tensor_scalar(
                                    out=gih[:],
                                    in0=iotg_sb[:, h * P:(h + 1) * P],
                                    scalar1=gid_sb[:, s:s + 1],
                                    scalar2=winv_sb[:, s:s + 1],
                                    op0=mybir.AluOpType.is_equal,
                                    op1=mybir.AluOpType.mult)
                                nc.tensor.matmul(
                                    out=pool_ps[h][:], lhsT=gih[:], rhs=x2b[:],
                                    start=(s == 0), stop=(s == n_slots - 1),
                                    skip_group_check=True)

                if not last:
                    nc.gpsimd.collective_compute(
                        "AllGather", mybir.AluOpType.bypass, replica_groups=rg,
                        ins=[T_sh[l + 1].ap().opt()],
                        outs=[T_full[l + 1].ap().opt()])

            # ---- head
            out_sb = spool.tile([P, G // P], F32, tag="outsb")
            for h in range(G // P):
                pool_sb = spool.tile([P, HID], F32, tag="poolsb",
                                     name=f"poolsb{h}")
                nc.vector.tensor_copy(out=pool_sb[:], in_=pool_ps[h][:])
                pt_ps = ppoolA.tile([HID, P], F32, tag="xtps")
                nc.tensor.transpose(out=pt_ps[:], in_=pool_sb[:],
                                    identity=ident_sb[:])
                pt_sb = spool.tile([HID, P], F32, tag="xtsb", name=f"ptsb{h}")
                nc.scalar.copy(out=pt_sb[:], in_=pt_ps[:])
                o_ps = ppoolA.tile([P, 1], F32, tag="tps", name=f"o_ps{h}")
                nc.tensor.matmul(out=o_ps[:], lhsT=pt_sb[:], rhs=wlin_sb[:],
                                 start=True, stop=True)
                nc.vector.tensor_copy(out=out_sb[:, h:h + 1], in_=o_ps[:])
            nc.sync.dma_start(out=out_d[:, :], in_=out_sb[:])

    nc.compile()
    return nc


# ----------------------------------------------------------------------------
# Entry point
# ----------------------------------------------------------------------------

def make_inmaps(pr: Prep, w: dict, T1, adcol1):
    identb = np.eye(P, dtype=np.float32).astype(BF)
    ident = np.eye(P, dtype=np.float32)
    iotg = np.tile(np.arange(G, dtype=np.float32)[None, :], (P, 1))
    in_maps = []
    for c in range(NCORES):
        m = {
            "T1": T1,
            "sidx": pr.sidx[c],
            "adcol1": adcol1[c],
            "gid": pr.gid[c],
            "winv": pr.winv[c],
            "wlin": w["wlin"],
            "identb": identb,
            "ident": ident,
            "iotg": iotg,
        }
        for li, l in enumerate((1, 2, 3)):
            m[f"attr{l}"] = pr.attr_m[li, c].astype(BF)
            m[f"brep{l}"] = w[f"brep{l}"]
        for l in (2, 3):
            m[f"wext{l}"] = w[f"wext{l}"]
        in_maps.append(m)
    return in_maps


def kernel(**inputs) -> np.ndarray:
    inputs = {k: np.asarray(v) for k, v in inputs.items()}
    w = pack_weights(inputs)
    pr = preprocess(inputs["edge_index"], inputs["edge_attr"], inputs["batch"],
                    [w["c1"], w["c2"], w["c3"]])
    T1, adcol1 = host_table1(pr, w, np.asarray(inputs["x"], np.float32))
    nc = build_program(pr)
    in_maps = make_inmaps(pr, w, T1, adcol1)
    res = bass_utils.run_bass_kernel_spmd(nc, in_maps,
                                          core_ids=list(range(NCORES)))
    out = np.zeros(G, np.float64)
    for c in range(NCORES):
        oc = res.results[c]["out"]
        out += oc.T.reshape(-1).astype(np.float64)
    return (out + w["blin"]).astype(np.float32)
